# revision 14
# baseline (speedup 1.0000x reference)
import sys
sys.path.insert(0, "/opt/trn_rl_repo")
import math
import numpy as np
import ml_dtypes

import concourse.bass as bass
from concourse import bacc, mybir
from concourse.tile import TileContext
from concourse.bass_utils import run_bass_kernel_spmd
from concourse.masks import make_identity

F32 = mybir.dt.float32
F32R = mybir.dt.float32r
F8 = mybir.dt.float8e4
AF = mybir.ActivationFunctionType
ALU = mybir.AluOpType
AX = mybir.AxisListType
DR = mybir.MatmulPerfMode.DoubleRow

N, G, E = 16384, 32, 524288
D, DFF, ZI, K, L = 512, 1024, 64, 4, 4
UMAP_A, UMAP_B = 1.577, 0.8951
BN_EPS = 1e-5
NCORES = 8
NL = N // NCORES      # 2048 local nodes per core
GL = G // NCORES      # 4 local graphs per core
NG = N // G           # 512 nodes per graph
NKP = N // 256        # 64 source k-pairs (256 rows each)
NCH = 4               # AllGather chunks (4096 table rows each)

_NC_CACHE = None


def build_nc():
    nc = bacc.Bacc("TRN2", target_bir_lowering=False, debug=False,
                   enable_asserts=True, num_devices=NCORES)

    xt = nc.dram_tensor("xt", (10, NL), F32R, kind="ExternalInput")
    acm8 = nc.dram_tensor("acm8", (N, NL), F8, kind="ExternalInput")
    embw = nc.dram_tensor("embw", (10, D), F32R, kind="ExternalInput")
    gw1 = nc.dram_tensor("gw1", (L * D, D), F32R, kind="ExternalInput")
    gw2 = nc.dram_tensor("gw2", (L * D, D), F32R, kind="ExternalInput")
    mw1 = nc.dram_tensor("mw1", (D, DFF), F32R, kind="ExternalInput")
    mw2 = nc.dram_tensor("mw2", (DFF, DFF), F32R, kind="ExternalInput")
    mw3 = nc.dram_tensor("mw3", (DFF, ZI), F32R, kind="ExternalInput")
    hw1 = nc.dram_tensor("hw1", (K * ZI, ZI), F32R, kind="ExternalInput")
    hw2 = nc.dram_tensor("hw2", (K * ZI, ZI), F32R, kind="ExternalInput")
    gb1_d = nc.dram_tensor("gb1_d", (128, 16), F32, kind="ExternalInput")
    bng_d = nc.dram_tensor("bng_d", (128, 16), F32, kind="ExternalInput")
    bnb_d = nc.dram_tensor("bnb_d", (128, 16), F32, kind="ExternalInput")
    mb1_d = nc.dram_tensor("mb1_d", (128, 8), F32, kind="ExternalInput")
    mb2_d = nc.dram_tensor("mb2_d", (128, 8), F32, kind="ExternalInput")
    mb3_d = nc.dram_tensor("mb3_d", (ZI, 1), F32, kind="ExternalInput")
    hb1_d = nc.dram_tensor("hb1_d", (ZI, K), F32, kind="ExternalInput")
    hb2_d = nc.dram_tensor("hb2_d", (ZI, K), F32, kind="ExternalInput")
    qout = nc.dram_tensor("qout", (GL * K * 4 * 128, NG), F32,
                          kind="ExternalOutput")

    with TileContext(nc) as tc:
        with (
            tc.tile_pool(name="const", bufs=1) as cp,
            tc.tile_pool(name="res", bufs=1) as rp,
            tc.tile_pool(name="ps", bufs=1, space="PSUM") as ps,
            tc.tile_pool(name="dram", bufs=1, space="DRAM") as dp,
        ):
            ident = cp.tile([128, 128], F32, tag="ident")
            make_identity(nc, ident[:])
            # exp path: t = exp(UMAP_B * ln(d2) + ln(UMAP_A)); q = 1/(1+t)
            pla = cp.tile([128, 1], F32, tag="pla")
            nc.gpsimd.memset(pla[:], math.log(UMAP_A))
            of = cp.tile([64, 1], F32, tag="of")
            nc.gpsimd.memset(of[:], 1.0)
            onf = cp.tile([1, NG], F32, tag="onf")
            nc.gpsimd.memset(onf[:], 1.0)

            gb1w = cp.tile([128, 16], F32, tag="gb1w")
            nc.sync.dma_start(gb1w[:], gb1_d[:, :])
            bngw = cp.tile([128, 16], F32, tag="bngw")
            nc.sync.dma_start(bngw[:], bng_d[:, :])
            bnbw = cp.tile([128, 16], F32, tag="bnbw")
            nc.sync.dma_start(bnbw[:], bnb_d[:, :])
            mb1w = cp.tile([128, 8], F32, tag="mb1w")
            nc.sync.dma_start(mb1w[:], mb1_d[:, :])
            mb2w = cp.tile([128, 8], F32, tag="mb2w")
            nc.sync.dma_start(mb2w[:], mb2_d[:, :])
            mb3w = cp.tile([ZI, 1], F32, tag="mb3w")
            nc.sync.dma_start(mb3w[:], mb3_d[:, :])
            hb1w = cp.tile([ZI, K], F32, tag="hb1w")
            nc.sync.dma_start(hb1w[:], hb1_d[:, :])
            hb2w = cp.tile([ZI, K], F32, tag="hb2w")
            nc.sync.dma_start(hb2w[:], hb2_d[:, :])

            hT = [rp.tile([128, NL], F32R, tag=f"hT{fc}", name=f"hT{fc}")
                  for fc in range(4)]

            # h tables in hi|lo fp8: slice rows are local nodes in natural
            # order; table rows are chunk-major: t = j*4096 + c*512 + r
            # <-> node c*2048 + j*512 + r.  acm8 rows are host-permuted to
            # match the table order.
            h8s = [dp.tile([NL, 1024], F8, tag=f"h8s{i}", name=f"h8s{i}")
                   for i in range(4)]
            # one Shared tensor per AllGather chunk (Shared DRAM allows only
            # a single writing instruction)
            h8t = [[dp.tile([4096, 1024], F8, tag=f"h8t{i}_{j}",
                            name=f"h8t{i}_{j}", addr_space="Shared")
                    for j in range(NCH)]
                   for i in range(4)]
            bn_loc = [dp.tile([128, 8], F32, tag=f"bl{i}", name=f"bl{i}")
                      for i in range(L)]
            dum_l = dp.tile([16, 4], F32, tag="dum_l", name="dum_l")
            dum_g = dp.tile([16, 4], F32, tag="dum_g", name="dum_g",
                            addr_space="Shared")
            bn_glob = [dp.tile([128, 8], F32, tag=f"bg{i}", name=f"bg{i}",
                               addr_space="Shared")
                       for i in range(L)]

            def ag_chunk(l, j):
                nc.gpsimd.collective_compute(
                    "AllGather", ALU.bypass,
                    ins=[h8s[l][512 * j:512 * j + 512, :].opt()],
                    outs=[h8t[l][j][:, :].opt()],
                    replica_groups=[list(range(NCORES))],
                )

            # warm up the collective stack while the embedding computes
            nc.gpsimd.collective_compute(
                "AllReduce", ALU.add,
                ins=[dum_l[:, :].opt()], outs=[dum_g[:, :].opt()],
                replica_groups=[list(range(NCORES))],
            )

            # ---------------- embedding ----------------
            with tc.tile_pool(name="emb", bufs=1) as ep:
                xt_sb = ep.tile([10, NL], F32R, tag="xt")
                nc.sync.dma_start(xt_sb[:], xt[:, :])
                ew_sb = ep.tile([10, D], F32R, tag="ew")
                nc.sync.dma_start(ew_sb[:], embw[:, :])
                for i in range(16):
                    p = ps.tile([128, 512], F32, tag=f"b{i % 4}")
                    nc.tensor.matmul(p[:], xt_sb[:, 128 * i:128 * i + 128],
                                     ew_sb[:], start=True, stop=True)
                    hi8 = ep.tile([128, 512], F8, tag="hi8", bufs=2)
                    nc.vector.tensor_copy(hi8[:], p[:])
                    lo8 = ep.tile([128, 512], F8, tag="lo8", bufs=2)
                    nc.vector.tensor_tensor(out=lo8[:], in0=p[:], in1=hi8[:],
                                            op=ALU.subtract)
                    nc.sync.dma_start(h8s[0][128 * i:128 * i + 128, 0:512],
                                      hi8[:])
                    nc.sync.dma_start(h8s[0][128 * i:128 * i + 128, 512:1024],
                                      lo8[:])
                    if i % 4 == 3:
                        ag_chunk(0, i // 4)
                for fc in range(4):
                    for j in range(4):
                        p = ps.tile([128, 512], F32, tag=f"b{4 + fc}")
                        nc.tensor.matmul(p[:], ew_sb[:, 128 * fc:128 * fc + 128],
                                         xt_sb[:, 512 * j:512 * j + 512],
                                         start=True, stop=True)
                        nc.vector.tensor_copy(hT[fc][:, 512 * j:512 * j + 512],
                                              p[:])

            # ---------------- GIN layers ----------------
            with tc.tile_pool(name="gin", bufs=1) as gp:
                for l in range(L):
                    w1s = gp.tile([128, 2048], F32R, tag="w1", bufs=2)
                    w2s = gp.tile([128, 2048], F32R, tag="w2", bufs=2)
                    for ic in range(4):
                        r0 = 512 * l + 128 * ic
                        nc.sync.dma_start(w1s[:, 512 * ic:512 * ic + 512],
                                          gw1[r0:r0 + 128, :])
                        nc.sync.dma_start(w2s[:, 512 * ic:512 * ic + 512],
                                          gw2[r0:r0 + 128, :])
                    mt = [gp.tile([128, NL], F32R, tag=f"mt{fc}", name=f"mt{fc}_{l}")
                          for fc in range(4)]
                    u2 = [gp.tile([128, NL], F32R, tag=f"u2_{fc}", name=f"u2_{fc}_{l}")
                          for fc in range(4)]

                    # aggregation: aggT += (h_hi + h_lo).T @ A  via fp8
                    # DoubleRow (2 contraction rows / cycle)
                    for half in range(2):
                        pb = [ps.tile([128, 512], F32, tag=f"b{i}", name=f"pb{i}")
                              for i in range(8)]
                        for kp in range(NKP):
                            cj, r0 = kp // 16, 256 * (kp % 16)
                            tab = h8t[l][cj]
                            hhi = gp.tile([128, 2, 512], F8, tag="hhi", bufs=3)
                            hlo = gp.tile([128, 2, 512], F8, tag="hlo", bufs=3)
                            ai8 = gp.tile([128, 2, 1024], F8, tag="ai", bufs=3)
                            for b in range(2):
                                rb = r0 + 128 * b
                                ra = 4096 * cj + rb
                                nc.sync.dma_start(hhi[:, b, :],
                                                  tab[rb:rb + 128, 0:512])
                                nc.sync.dma_start(hlo[:, b, :],
                                                  tab[rb:rb + 128, 512:1024])
                                nc.sync.dma_start(
                                    ai8[:, b, :],
                                    acm8[ra:ra + 128,
                                         1024 * half:1024 * half + 1024])
                            for fc in range(4):
                                for dc in range(2):
                                    bank = pb[fc * 2 + dc]
                                    nc.tensor.matmul(
                                        bank[:],
                                        hhi[:, :, 128 * fc:128 * fc + 128],
                                        ai8[:, :, 512 * dc:512 * dc + 512],
                                        start=(kp == 0), stop=False,
                                        perf_mode=DR)
                                    nc.tensor.matmul(
                                        bank[:],
                                        hlo[:, :, 128 * fc:128 * fc + 128],
                                        ai8[:, :, 512 * dc:512 * dc + 512],
                                        start=False, stop=(kp == NKP - 1),
                                        perf_mode=DR)
                        for fc in range(4):
                            for dc in range(2):
                                col = 1024 * half + 512 * dc
                                nc.vector.tensor_tensor(
                                    out=mt[fc][:, col:col + 512],
                                    in0=pb[fc * 2 + dc][:],
                                    in1=hT[fc][:, col:col + 512],
                                    op=ALU.add)

                    # GIN MLP: u1 = relu(m@w1+b1); u2 = u1@w2
                    for j in range(4):
                        ncol = 512 * j
                        u1c = [gp.tile([128, 512], F32R, tag=f"u1_{oc}", bufs=2,
                                        name=f"u1c{oc}") for oc in range(4)]
                        for oc in range(4):
                            p = ps.tile([128, 512], F32, tag=f"b{oc}")
                            for ic in range(4):
                                nc.tensor.matmul(
                                    p[:],
                                    w1s[:, 512 * ic + 128 * oc:
                                        512 * ic + 128 * oc + 128],
                                    mt[ic][:, ncol:ncol + 512],
                                    start=(ic == 0), stop=(ic == 3))
                            nc.scalar.activation(
                                u1c[oc][:], p[:], AF.Relu,
                                bias=gb1w[:, 4 * l + oc:4 * l + oc + 1])
                        for oc in range(4):
                            p = ps.tile([128, 512], F32, tag=f"b{4 + oc}")
                            for ic in range(4):
                                nc.tensor.matmul(
                                    p[:],
                                    w2s[:, 512 * ic + 128 * oc:
                                        512 * ic + 128 * oc + 128],
                                    u1c[ic][:],
                                    start=(ic == 0), stop=(ic == 3))
                            nc.vector.tensor_copy(u2[oc][:, ncol:ncol + 512],
                                                  p[:])

                    # BN stats (local sums + sumsq) -> AllReduce
                    stat = gp.tile([128, 8], F32, tag="stat")
                    sqs = gp.tile([128, NL], F32, tag="sqs")
                    for fc in range(4):
                        nc.vector.reduce_sum(stat[:, fc:fc + 1], u2[fc][:],
                                             axis=AX.X)
                        nc.scalar.activation(sqs[:], u2[fc][:], AF.Square,
                                             accum_out=stat[:, 4 + fc:5 + fc])
                    nc.sync.dma_start(bn_loc[l][:, :], stat[:])
                    nc.gpsimd.collective_compute(
                        "AllReduce", ALU.add,
                        ins=[bn_loc[l][:, :].opt()],
                        outs=[bn_glob[l][:, :].opt()],
                        replica_groups=[list(range(NCORES))],
                    )
                    ga = gp.tile([128, 8], F32, tag="ga")
                    nc.sync.dma_start(ga[:], bn_glob[l][:, :])

                    # batched BN scalars for all 4 fc at once
                    mu4 = gp.tile([128, 4], F32, tag="mu4")
                    nc.vector.tensor_scalar(out=mu4[:], in0=ga[:, 0:4],
                                            scalar1=1.0 / N, scalar2=None,
                                            op0=ALU.mult)
                    ex24 = gp.tile([128, 4], F32, tag="ex24")
                    nc.vector.tensor_scalar(out=ex24[:], in0=ga[:, 4:8],
                                            scalar1=1.0 / N, scalar2=None,
                                            op0=ALU.mult)
                    mu24 = gp.tile([128, 4], F32, tag="mu24")
                    nc.vector.tensor_tensor(out=mu24[:], in0=mu4[:],
                                            in1=mu4[:], op=ALU.mult)
                    var4 = gp.tile([128, 4], F32, tag="var4")
                    nc.vector.tensor_tensor(out=var4[:], in0=ex24[:],
                                            in1=mu24[:], op=ALU.subtract)
                    vare4 = gp.tile([128, 4], F32, tag="vare4")
                    nc.vector.tensor_scalar(out=vare4[:], in0=var4[:],
                                            scalar1=BN_EPS, scalar2=None,
                                            op0=ALU.add)
                    std4 = gp.tile([128, 4], F32, tag="std4")
                    nc.scalar.activation(std4[:], vare4[:], AF.Sqrt)
                    inv4 = gp.tile([128, 4], F32, tag="inv4")
                    nc.vector.reciprocal(inv4[:], std4[:])
                    sv4 = gp.tile([128, 4], F32, tag="sv4")
                    nc.vector.tensor_tensor(
                        out=sv4[:], in0=inv4[:],
                        in1=bngw[:, 4 * l:4 * l + 4], op=ALU.mult)
                    mst4 = gp.tile([128, 4], F32, tag="mst4")
                    nc.vector.tensor_tensor(out=mst4[:], in0=mu4[:],
                                            in1=sv4[:], op=ALU.mult)
                    tv4 = gp.tile([128, 4], F32, tag="tv4")
                    nc.vector.tensor_tensor(
                        out=tv4[:], in0=bnbw[:, 4 * l:4 * l + 4],
                        in1=mst4[:], op=ALU.subtract)

                    # BN apply + relu + residual, then (l<3) cast hi/lo fp8,
                    # transpose to node-major and AllGather per node chunk
                    for j in range(4):
                        ncol = 512 * j
                        for fc in range(4):
                            rt = gp.tile([128, 512], F32R, tag="rt", bufs=3)
                            nc.scalar.activation(
                                rt[:], u2[fc][:, ncol:ncol + 512], AF.Relu,
                                bias=tv4[:, fc:fc + 1], scale=sv4[:, fc:fc + 1])
                            nc.vector.tensor_tensor(
                                out=hT[fc][:, ncol:ncol + 512], in0=rt[:],
                                in1=hT[fc][:, ncol:ncol + 512], op=ALU.add)
                        if l < L - 1:
                            for nb in range(4):
                                n0 = 4 * j + nb
                                hn2 = gp.tile([128, 512], F32R, tag="hn2",
                                              bufs=3)
                                for fc in range(4):
                                    pt = ps.tile([128, 128], F32, tag=f"b{fc}")
                                    nc.tensor.transpose(
                                        pt[:],
                                        hT[fc][:, 128 * n0:128 * n0 + 128]
                                        .bitcast(F32),
                                        ident[:])
                                    nc.vector.tensor_copy(
                                        hn2[:, 128 * fc:128 * fc + 128], pt[:])
                                hi8 = gp.tile([128, 512], F8, tag="hi8",
                                              bufs=3)
                                nc.vector.tensor_copy(hi8[:], hn2[:])
                                lo8 = gp.tile([128, 512], F8, tag="lo8",
                                              bufs=3)
                                nc.vector.tensor_tensor(
                                    out=lo8[:], in0=hn2[:], in1=hi8[:],
                                    op=ALU.subtract)
                                nc.sync.dma_start(
                                    h8s[l + 1][128 * n0:128 * n0 + 128, 0:512],
                                    hi8[:])
                                nc.sync.dma_start(
                                    h8s[l + 1][128 * n0:128 * n0 + 128,
                                               512:1024],
                                    lo8[:])
                            ag_chunk(l + 1, j)

            # ---------------- final MLP + heads + pairwise ----------------
            with tc.tile_pool(name="fin", bufs=1) as fp:
                mwa = [fp.tile([128, DFF], F32R, tag=f"mw1_{ic}", name=f"mwa{ic}")
                       for ic in range(4)]
                for ic in range(4):
                    nc.sync.dma_start(mwa[ic][:],
                                      mw1[128 * ic:128 * ic + 128, :])
                mwb = [fp.tile([128, DFF], F32R, tag=f"mw2_{ic}", name=f"mwb{ic}")
                       for ic in range(8)]
                for ic in range(8):
                    nc.sync.dma_start(mwb[ic][:],
                                      mw2[128 * ic:128 * ic + 128, :])
                mwc = [fp.tile([128, ZI], F32R, tag=f"mw3_{ic}", name=f"mwc{ic}")
                       for ic in range(8)]
                for ic in range(8):
                    nc.sync.dma_start(mwc[ic][:],
                                      mw3[128 * ic:128 * ic + 128, :])
                hw1s = [fp.tile([ZI, ZI], F32R, tag=f"hw1_{k}", name=f"hw1s{k}")
                        for k in range(K)]
                hw2s = [fp.tile([ZI, ZI], F32R, tag=f"hw2_{k}", name=f"hw2s{k}")
                        for k in range(K)]
                for k in range(K):
                    nc.sync.dma_start(hw1s[k][:], hw1[ZI * k:ZI * k + ZI, :])
                    nc.sync.dma_start(hw2s[k][:], hw2[ZI * k:ZI * k + ZI, :])

                for g in range(GL):
                    gcol = 512 * g
                    z1 = [fp.tile([128, 512], F32R, tag=f"z1_{oc}", name=f"z1_{oc}")
                          for oc in range(8)]
                    for oc in range(8):
                        p = ps.tile([128, 512], F32, tag=f"b{oc}")
                        for ic in range(4):
                            nc.tensor.matmul(
                                p[:],
                                mwa[ic][:, 128 * oc:128 * oc + 128],
                                hT[ic][:, gcol:gcol + 512],
                                start=(ic == 0), stop=(ic == 3))
                        nc.scalar.activation(z1[oc][:], p[:], AF.Relu,
                                             bias=mb1w[:, oc:oc + 1])
                    z2 = [fp.tile([128, 512], F32R, tag=f"z2_{oc}", name=f"z2_{oc}")
                          for oc in range(8)]
                    for oc in range(8):
                        p = ps.tile([128, 512], F32, tag=f"b{oc}")
                        for ic in range(8):
                            nc.tensor.matmul(
                                p[:],
                                mwb[ic][:, 128 * oc:128 * oc + 128],
                                z1[ic][:],
                                start=(ic == 0), stop=(ic == 7))
                        nc.scalar.activation(z2[oc][:], p[:], AF.Relu,
                                             bias=mb2w[:, oc:oc + 1])
                    pz = ps.tile([ZI, 512], F32, tag="b0")
                    for ic in range(8):
                        nc.tensor.matmul(pz[:], mwc[ic][:, 0:ZI], z2[ic][:],
                                         start=(ic == 0), stop=(ic == 7))
                    z3 = fp.tile([ZI, 512], F32R, tag="z3")
                    nc.vector.tensor_tensor(
                        out=z3[:], in0=pz[:],
                        in1=mb3w[:, 0:1].to_broadcast([ZI, 512])[:],
                        op=ALU.add)
                    for k in range(K):
                        p1 = ps.tile([ZI, 512], F32, tag=f"b{2 * (k % 2)}")
                        nc.tensor.matmul(p1[:], hw1s[k][:], z3[:],
                                         start=True, stop=True)
                        h1 = fp.tile([ZI, 512], F32R, tag="h1", bufs=3)
                        nc.scalar.activation(h1[:], p1[:], AF.Relu,
                                             bias=hb1w[:, k:k + 1])
                        p2 = ps.tile([ZI, 512], F32, tag=f"b{2 * (k % 2) + 1}")
                        nc.tensor.matmul(p2[:], hw2s[k][:], h1[:],
                                         start=True, stop=True)
                        hkt = fp.tile([ZI, 512], F32, tag="hkt", bufs=3)
                        nc.scalar.activation(hkt[:], p2[:], AF.Identity,
                                             bias=hb2w[:, k:k + 1])
                        hm2 = fp.tile([ZI, 512], F32, tag="hm2", bufs=2)
                        nc.scalar.activation(hm2[:], hkt[:], AF.Copy,
                                             scale=-2.0)
                        sqt = fp.tile([ZI, 512], F32, tag="sqt", bufs=2)
                        nc.scalar.activation(sqt[:], hkt[:], AF.Square)
                        pr = ps.tile([1, 512], F32, tag=f"b{2 * (k % 2)}")
                        nc.tensor.matmul(pr[:], of[:], sqt[:],
                                         start=True, stop=True)
                        rsb = fp.tile([1, 512], F32, tag="rsb", bufs=3)
                        nc.vector.tensor_copy(rsb[:], pr[:])
                        for mb in range(4):
                            pd = ps.tile([128, 512], F32, tag=f"b{4 + mb}")
                            nc.tensor.matmul(pd[:],
                                             hm2[:, 128 * mb:128 * mb + 128],
                                             hkt[:], start=True, stop=False)
                            nc.tensor.matmul(pd[:], onf[:, 0:128], rsb[:],
                                             start=False, stop=False,
                                             skip_group_check=True)
                            nc.tensor.matmul(pd[:],
                                             rsb[:, 128 * mb:128 * mb + 128],
                                             onf[:], start=False, stop=True,
                                             skip_group_check=True)
                            d2t = fp.tile([128, 512], F32, tag="d2", bufs=3)
                            nc.vector.tensor_scalar(out=d2t[:], in0=pd[:],
                                                    scalar1=1e-12,
                                                    scalar2=None, op0=ALU.max)
                            lnt = fp.tile([128, 512], F32, tag="ln", bufs=3)
                            nc.scalar.activation(lnt[:], d2t[:], AF.Ln)
                            et = fp.tile([128, 512], F32, tag="et", bufs=3)
                            nc.scalar.activation(et[:], lnt[:], AF.Exp,
                                                 bias=pla[:, 0:1],
                                                 scale=UMAP_B)
                            e1t = fp.tile([128, 512], F32, tag="e1t", bufs=3)
                            nc.vector.tensor_scalar(out=e1t[:], in0=et[:],
                                                    scalar1=1.0,
                                                    scalar2=None, op0=ALU.add)
                            qt = fp.tile([128, 512], F32, tag="qt", bufs=3)
                            nc.vector.reciprocal(qt[:], e1t[:])
                            row = ((g * K + k) * 4 + mb) * 128
                            nc.sync.dma_start(qout[row:row + 128, :], qt[:])
    nc.compile()
    return nc


def _host_prep(inputs):
    x = np.asarray(inputs["x"], np.float32)
    edge_index = np.asarray(inputs["edge_index"], np.int64)
    src, dst = edge_index[0], edge_index[1]

    # device keeps hT' = 16*h so the fp8 hi/lo split of h stays out of the
    # e4m3 subnormal range; all compensation is folded into host weights:
    # embw*16 (h0'), gin_w1/16 (consumes 16*m), bn_g*16 & bn_b*16 (produce
    # 16*relu(bn)), mlp_w1/16 (consumes 16*h)
    HS = 16.0
    shared = {
        "embw": np.ascontiguousarray(HS * np.vstack(
            [np.asarray(inputs["emb_w"], np.float32),
             np.asarray(inputs["emb_b"], np.float32)[None, :]])),
        "gw1": np.ascontiguousarray(
            np.asarray(inputs["gin_w1"], np.float32).reshape(L * D, D) / HS),
        "gw2": np.ascontiguousarray(
            np.asarray(inputs["gin_w2"], np.float32).reshape(L * D, D)),
        "mw1": np.ascontiguousarray(
            np.asarray(inputs["mlp_w1"], np.float32) / HS),
        "mw2": np.ascontiguousarray(np.asarray(inputs["mlp_w2"], np.float32)),
        "mw3": np.ascontiguousarray(np.asarray(inputs["mlp_w3"], np.float32)),
        "hw1": np.ascontiguousarray(
            np.asarray(inputs["head_w1"], np.float32).reshape(K * ZI, ZI)),
        "hw2": np.ascontiguousarray(
            np.asarray(inputs["head_w2"], np.float32).reshape(K * ZI, ZI)),
        "gb1_d": np.ascontiguousarray(
            np.asarray(inputs["gin_b1"], np.float32)
            .reshape(L, 4, 128).transpose(2, 0, 1).reshape(128, 16)),
        "bng_d": np.ascontiguousarray(
            HS * np.asarray(inputs["bn_g"], np.float32)
            .reshape(L, 4, 128).transpose(2, 0, 1).reshape(128, 16)),
        "bnb_d": np.ascontiguousarray(
            HS * np.asarray(inputs["bn_b"], np.float32)
            .reshape(L, 4, 128).transpose(2, 0, 1).reshape(128, 16)),
        "mb1_d": np.ascontiguousarray(
            np.asarray(inputs["mlp_b1"], np.float32).reshape(8, 128).T),
        "mb2_d": np.ascontiguousarray(
            np.asarray(inputs["mlp_b2"], np.float32).reshape(8, 128).T),
        "mb3_d": np.ascontiguousarray(
            np.asarray(inputs["mlp_b3"], np.float32)[:, None]),
        "hb1_d": np.ascontiguousarray(
            np.asarray(inputs["head_b1"], np.float32).T),
        "hb2_d": np.ascontiguousarray(
            np.asarray(inputs["head_b2"], np.float32).T),
    }

    # table-row -> natural-node permutation: t = j*4096 + c*512 + r maps to
    # node c*2048 + j*512 + r   (j: AG chunk, c: source core, r: row)
    jj = np.repeat(np.arange(NCH), NCORES * 512)
    cc = np.tile(np.repeat(np.arange(NCORES), 512), NCH)
    rr = np.tile(np.arange(512), NCH * NCORES)
    perm = cc * NL + jj * 512 + rr

    in_maps = []
    ones_row = np.ones((1, NL), np.float32)
    for c in range(NCORES):
        lo = NL * c
        mask = (dst >= lo) & (dst < lo + NL)
        flat = src[mask] * NL + (dst[mask] - lo)
        a = np.bincount(flat, minlength=N * NL).astype(np.uint8)
        a = a.reshape(N, NL)
        m = dict(shared)
        m["acm8"] = np.ascontiguousarray(
            a[perm].astype(ml_dtypes.float8_e4m3))
        m["xt"] = np.ascontiguousarray(
            np.vstack([x[lo:lo + NL].T, ones_row]))
        in_maps.append(m)
    return in_maps


def kernel(**inputs) -> np.ndarray:
    global _NC_CACHE
    if _NC_CACHE is None:
        _NC_CACHE = build_nc()
    nc = _NC_CACHE
    in_maps = _host_prep(inputs)
    res = run_bass_kernel_spmd(nc, in_maps, core_ids=list(range(NCORES)))
    out = np.concatenate(
        [np.asarray(res.results[c]["qout"]).reshape(GL, K, NG, NG)
         for c in range(NCORES)], axis=0)
    return out
